# revision 1
# baseline (speedup 1.0000x reference)
"""MoE (8 experts, top-2) Trainium2 Bass kernel, expert-parallel over 8 cores.

Pipeline (all FLOPs on device):
  L1: gating logits for all tokens (data-parallel over cores)
  host: top-2 selection + per-expert dispatch lists (comparisons/indexing only)
  L2: per-core expert MLP (3 layers, fp32) on gathered tokens, feature-major
  L3: per-token gather of its two expert rows + on-device softmax combine
"""

import numpy as np

import jax

jax.config.update("jax_compilation_cache_dir", "/tmp/jax_comp_cache")
jax.config.update("jax_persistent_cache_min_entry_size_bytes", -1)
jax.config.update("jax_persistent_cache_min_compile_time_secs", 0)

import concourse.bass as bass
import concourse.mybir as mybir
import concourse.tile as tile
from concourse import bacc
from concourse.bass_utils import run_bass_kernel_spmd
from concourse.kernels.tile_matmul import matmul_tile_kernel

N, D, H, O, E = 8192, 1024, 2048, 1024, 8
NCORES = 8
TPC = N // NCORES  # tokens per core (gating / combine phases)
F32 = mybir.dt.float32

_CACHE = {}


def _to_pmn(a):
    """[K, N] row-major -> [128, K/128, N] with row k = m*128 + p."""
    K, Nn = a.shape
    return np.ascontiguousarray(a.reshape(K // 128, 128, Nn).transpose(1, 0, 2))


def _nc():
    return bacc.Bacc(None, target_bir_lowering=False, debug=True)


def _build_gate_nc():
    nc = _nc()
    xt = nc.dram_tensor("xt", [128, D // 128, TPC], F32, kind="ExternalInput")
    wg1 = nc.dram_tensor("wg1", [128, D // 128, 128], F32, kind="ExternalInput")
    wg2 = nc.dram_tensor("wg2", [128, 1, 128], F32, kind="ExternalInput")
    g1t = nc.dram_tensor("g1t", [128, 1, TPC], F32)
    logt = nc.dram_tensor("logt", [128, 1, TPC], F32, kind="ExternalOutput")
    with tile.TileContext(nc) as tc:
        matmul_tile_kernel(tc, wg1[:], xt[:], g1t[:], use_relu=True)
        matmul_tile_kernel(tc, wg2[:], g1t[:], logt[:])
    nc.compile()
    return nc


def _build_mlp_nc(C):
    # Matmuls in bf16 (1 cycle/row on PE vs 4 for strict fp32), fp32 PSUM
    # accumulate. Weights/x cast to bf16 on device per tile; h1/h2 stored
    # bf16 (halves intermediate HBM traffic), final output fp32.
    BF = mybir.dt.bfloat16
    nc = _nc()
    xt = nc.dram_tensor("xt", [128, D // 128, C], F32, kind="ExternalInput")
    w1 = nc.dram_tensor("w1", [128, D // 128, H], F32, kind="ExternalInput")
    w2 = nc.dram_tensor("w2", [128, H // 128, H], F32, kind="ExternalInput")
    w3 = nc.dram_tensor("w3", [128, H // 128, O], F32, kind="ExternalInput")
    h1 = nc.dram_tensor("h1", [128, H // 128, C], BF)
    h2 = nc.dram_tensor("h2", [128, H // 128, C], BF)
    yt = nc.dram_tensor("yt", [128, O // 128, C], F32, kind="ExternalOutput")
    with tile.TileContext(nc) as tc:
        matmul_tile_kernel(tc, w1[:], xt[:], h1[:], use_relu=True, matmul_dtype=BF)
        matmul_tile_kernel(tc, w2[:], h1[:], h2[:], use_relu=True, matmul_dtype=BF)
        matmul_tile_kernel(tc, w3[:], h2[:], yt[:], matmul_dtype=BF)
    nc.compile()
    return nc


def _build_mlp2_nc(S):
    # Two independent expert-segment slots in one module: one launch does
    # the work of two, paying the kernel tail barrier/warmup once.
    BF = mybir.dt.bfloat16
    nc = _nc()
    t = {}
    for s in ("A", "B"):
        t[f"xt{s}"] = nc.dram_tensor(f"xt{s}", [128, D // 128, S], F32, kind="ExternalInput")
        t[f"w1{s}"] = nc.dram_tensor(f"w1{s}", [128, D // 128, H], F32, kind="ExternalInput")
        t[f"w2{s}"] = nc.dram_tensor(f"w2{s}", [128, H // 128, H], F32, kind="ExternalInput")
        t[f"w3{s}"] = nc.dram_tensor(f"w3{s}", [128, H // 128, O], F32, kind="ExternalInput")
        t[f"h1{s}"] = nc.dram_tensor(f"h1{s}", [128, H // 128, S], BF)
        t[f"h2{s}"] = nc.dram_tensor(f"h2{s}", [128, H // 128, S], BF)
        t[f"yt{s}"] = nc.dram_tensor(f"yt{s}", [128, O // 128, S], F32, kind="ExternalOutput")
    with tile.TileContext(nc) as tc:
        for s in ("A", "B"):
            matmul_tile_kernel(tc, t[f"w1{s}"][:], t[f"xt{s}"][:], t[f"h1{s}"][:], use_relu=True, matmul_dtype=BF)
            matmul_tile_kernel(tc, t[f"w2{s}"][:], t[f"h1{s}"][:], t[f"h2{s}"][:], use_relu=True, matmul_dtype=BF)
            matmul_tile_kernel(tc, t[f"w3{s}"][:], t[f"h2{s}"][:], t[f"yt{s}"][:], matmul_dtype=BF)
    nc.compile()
    return nc


def _build_combine_nc(yall_rows):
    nc = _nc()
    ntiles = TPC // 128
    yall = nc.dram_tensor("yall", [yall_rows, O], F32, kind="ExternalInput")
    lg = nc.dram_tensor("lg", [128, ntiles, E], F32, kind="ExternalInput")
    m0 = nc.dram_tensor("m0", [128, ntiles, E], F32, kind="ExternalInput")
    m1 = nc.dram_tensor("m1", [128, ntiles, E], F32, kind="ExternalInput")
    i0 = nc.dram_tensor("i0", [128, ntiles], mybir.dt.int32, kind="ExternalInput")
    i1 = nc.dram_tensor("i1", [128, ntiles], mybir.dt.int32, kind="ExternalInput")
    out = nc.dram_tensor("out", [128, ntiles, O], F32, kind="ExternalOutput")
    X = mybir.AxisListType.X
    with tile.TileContext(nc) as tc:
        with (
            tc.tile_pool(name="big", bufs=4) as gp,
            tc.tile_pool(name="small", bufs=8) as sp,
            tc.tile_pool(name="idx", bufs=4) as ip,
        ):
            for i in range(ntiles):
                lg_t = sp.tile([128, E], F32, tag="lg")
                m0_t = sp.tile([128, E], F32, tag="m0")
                m1_t = sp.tile([128, E], F32, tag="m1")
                i0_t = ip.tile([128, 1], mybir.dt.int32, tag="i0")
                i1_t = ip.tile([128, 1], mybir.dt.int32, tag="i1")
                nc.sync.dma_start(lg_t[:], lg[:, i, :])
                nc.sync.dma_start(m0_t[:], m0[:, i, :])
                nc.sync.dma_start(m1_t[:], m1[:, i, :])
                nc.sync.dma_start(i0_t[:], i0[:, i : i + 1])
                nc.sync.dma_start(i1_t[:], i1[:, i : i + 1])

                g0 = gp.tile([128, O], F32, tag="g0")
                g1 = gp.tile([128, O], F32, tag="g1")
                nc.gpsimd.indirect_dma_start(
                    out=g0[:],
                    out_offset=None,
                    in_=yall[:],
                    in_offset=bass.IndirectOffsetOnAxis(ap=i0_t[:, :1], axis=0),
                )
                nc.gpsimd.indirect_dma_start(
                    out=g1[:],
                    out_offset=None,
                    in_=yall[:],
                    in_offset=bass.IndirectOffsetOnAxis(ap=i1_t[:, :1], axis=0),
                )

                rmax = sp.tile([128, 1], F32, tag="rmax")
                nc.vector.reduce_max(out=rmax[:], in_=lg_t[:], axis=X)
                ex = sp.tile([128, E], F32, tag="ex")
                nc.vector.tensor_scalar_sub(out=ex[:], in0=lg_t[:], scalar1=rmax[:])
                nc.scalar.activation(
                    out=ex[:], in_=ex[:], func=mybir.ActivationFunctionType.Exp
                )
                s = sp.tile([128, 1], F32, tag="s")
                nc.vector.reduce_sum(out=s[:], in_=ex[:], axis=X)
                inv = sp.tile([128, 1], F32, tag="inv")
                nc.vector.reciprocal(out=inv[:], in_=s[:])

                t0 = sp.tile([128, E], F32, tag="t0")
                nc.vector.tensor_mul(out=t0[:], in0=ex[:], in1=m0_t[:])
                w0 = sp.tile([128, 1], F32, tag="w0")
                nc.vector.reduce_sum(out=w0[:], in_=t0[:], axis=X)
                nc.vector.tensor_mul(out=w0[:], in0=w0[:], in1=inv[:])

                t1 = sp.tile([128, E], F32, tag="t1")
                nc.vector.tensor_mul(out=t1[:], in0=ex[:], in1=m1_t[:])
                w1v = sp.tile([128, 1], F32, tag="w1")
                nc.vector.reduce_sum(out=w1v[:], in_=t1[:], axis=X)
                nc.vector.tensor_mul(out=w1v[:], in0=w1v[:], in1=inv[:])

                nc.vector.tensor_scalar_mul(out=g0[:], in0=g0[:], scalar1=w0[:])
                nc.vector.tensor_scalar_mul(out=g1[:], in0=g1[:], scalar1=w1v[:])
                nc.vector.tensor_add(out=g0[:], in0=g0[:], in1=g1[:])
                nc.sync.dma_start(out[:, i, :], g0[:])
    nc.compile()
    return nc


def kernel(x, W1, b1, W2, b2, W3, b3, Wg1, bg1, Wg2, bg2, top_k):
    x = np.asarray(x, np.float32)
    W1 = np.asarray(W1, np.float32)
    W2 = np.asarray(W2, np.float32)
    W3 = np.asarray(W3, np.float32)
    Wg1 = np.asarray(Wg1, np.float32)
    Wg2 = np.asarray(Wg2, np.float32)
    assert int(np.asarray(top_k)) == 2
    for b in (b1, b2, b3, bg1, bg2):
        assert not np.any(np.asarray(b)), "nonzero biases unsupported"

    core_ids = list(range(NCORES))

    # ---------------- L1: gating logits on device ----------------
    if "gate" not in _CACHE:
        _CACHE["gate"] = _build_gate_nc()
    nc1 = _CACHE["gate"]

    xT = np.ascontiguousarray(x.T)  # [D, N]
    wg1p = np.zeros((D, 128), np.float32)
    wg1p[:, :64] = Wg1
    wg2p = np.zeros((128, 128), np.float32)
    wg2p[:64, :E] = Wg2
    wg1_pmn = _to_pmn(wg1p)
    wg2_pmn = _to_pmn(wg2p)
    in1 = [
        {
            "xt": _to_pmn(xT[:, c * TPC : (c + 1) * TPC]),
            "wg1": wg1_pmn,
            "wg2": wg2_pmn,
        }
        for c in core_ids
    ]
    res1 = run_bass_kernel_spmd(nc1, in1, core_ids).results
    logits = np.concatenate(
        [res1[c]["logt"][:E, 0, :].T for c in core_ids], axis=0
    )  # [N, E]

    # ---------------- host routing (comparisons/indexing only) ----------------
    top2 = np.argsort(-logits, axis=1, kind="stable")[:, :2]  # [N, 2]
    e0s, e1s = top2[:, 0], top2[:, 1]
    expert_lists = [np.nonzero((top2 == e).any(axis=1))[0] for e in range(E)]
    counts = np.array([len(t) for t in expert_lists])

    # Balanced segmentation: chop each expert's tokens into <=S chunks, 16
    # uniform slots total, run the 8-core MLP module twice (8 slots/launch).
    NSEG = 2 * NCORES
    S = max(128, -(-int(counts.sum()) // (NSEG * 128)) * 128)
    while sum(-(-c // S) for c in counts if c) > NSEG:
        S += 128
    segs = []  # (expert, token_array)
    for e in range(E):
        tl = expert_lists[e]
        for j in range(0, max(len(tl), 1), S):
            segs.append((e, tl[j : j + S]))
    while len(segs) < NSEG:
        segs.append((0, np.zeros(0, np.int64)))

    # token -> global row in yall: segment k occupies rows [k*S, k*S+len_k)
    seg_of_expert = {}  # (expert, chunk_idx) -> seg index
    for k, (e, tl) in enumerate(segs):
        if len(tl):
            seg_of_expert.setdefault(e, []).append(k)
    pos = np.zeros((N, E), np.int64)
    for e in range(E):
        pos[expert_lists[e], e] = np.arange(len(expert_lists[e]))

    def glob_idx(es):
        p = pos[np.arange(N), es]
        segids = np.array(
            [seg_of_expert[e][pp // S] for e, pp in zip(es, p)], np.int64
        )
        return (segids * S + (p % S)).astype(np.int32)

    glob0, glob1 = glob_idx(e0s), glob_idx(e1s)
    eye = np.eye(E, dtype=np.float32)

    # ---------------- L2: per-expert MLP on device (2 launches) ----------------
    key2 = ("mlp", S)

    def seg_inputs(k):
        e, tl = segs[k]
        padded = np.zeros(S, np.int64)
        padded[: len(tl)] = tl
        Xe = x[padded]  # [S, D]
        return {
            "xt": _to_pmn(np.ascontiguousarray(Xe.T)),
            "w1": _to_pmn(W1[e]),
            "w2": _to_pmn(W2[e]),
            "w3": _to_pmn(W3[e]),
        }

    yall = np.empty((NSEG * S, O), np.float32)
    try:
        key2f = ("mlp2", S)
        if key2f not in _CACHE:
            _CACHE[key2f] = _build_mlp2_nc(S)
        in2 = []
        for c in core_ids:
            a, b = seg_inputs(c), seg_inputs(NCORES + c)
            in2.append(
                {**{k + "A": v for k, v in a.items()}, **{k + "B": v for k, v in b.items()}}
            )
        res2 = run_bass_kernel_spmd(_CACHE[key2f], in2, core_ids).results
        for c in core_ids:
            for half, s in enumerate(("A", "B")):
                k = half * NCORES + c
                yT = res2[c][f"yt{s}"].transpose(1, 0, 2).reshape(O, S)
                yall[k * S : (k + 1) * S] = yT.T
    except Exception:
        if key2 not in _CACHE:
            _CACHE[key2] = _build_mlp_nc(S)
        nc2 = _CACHE[key2]
        for half in range(2):
            in2 = [seg_inputs(half * NCORES + c) for c in core_ids]
            res2 = run_bass_kernel_spmd(nc2, in2, core_ids).results
            for c in core_ids:
                k = half * NCORES + c
                yT = res2[c]["yt"].transpose(1, 0, 2).reshape(O, S)
                yall[k * S : (k + 1) * S] = yT.T

    # ---------------- L3: gather + softmax combine on device ----------------
    key3 = ("comb", NSEG * S)
    if key3 not in _CACHE:
        _CACHE[key3] = _build_combine_nc(NSEG * S)
    nc3 = _CACHE[key3]

    ntiles = TPC // 128

    def _pt(a):  # [TPC, ...] -> [128, ntiles, ...] with token = i*128 + p
        return np.ascontiguousarray(
            a.reshape(ntiles, 128, *a.shape[1:]).transpose(1, 0, *range(2, a.ndim + 1))
        )

    in3 = []
    for c in core_ids:
        sl = slice(c * TPC, (c + 1) * TPC)
        in3.append(
            {
                "yall": yall,
                "lg": _pt(logits[sl]),
                "m0": _pt(eye[e0s[sl]]),
                "m1": _pt(eye[e1s[sl]]),
                "i0": _pt(glob0[sl].reshape(TPC, 1))[:, :, 0],
                "i1": _pt(glob1[sl].reshape(TPC, 1))[:, :, 0],
            }
        )
    res3 = run_bass_kernel_spmd(nc3, in3, core_ids).results
    out = np.concatenate(
        [res3[c]["out"].transpose(1, 0, 2).reshape(TPC, O) for c in core_ids], axis=0
    )
    return out



# revision 5
# speedup vs baseline: 1.4380x; 1.4380x over previous
"""MoE (8 experts, top-2) Trainium2 Bass kernel, expert-parallel over 8 cores.

Pipeline (all FLOPs on device):
  L1: gating logits for all tokens (data-parallel, fp32r matmul: fp32
      precision at 1 cycle/row so top-2 selection matches the reference)
  host: top-2 selection + per-expert dispatch lists (comparisons/indexing)
  L2: expert MLP in bf16. Instead of 16 uniform padded segments, the
      module is built at runtime with 3 chains per core whose sizes are
      chosen by bin-packing the actual per-expert token counts
      (8 copies of each size across cores; every chain carries its own
      expert's weights as per-core inputs). This cuts padding from ~37%
      to ~2%.
  L3: per-token gather of its two expert rows (bf16) + on-device softmax
      combine in fp32.
"""

import itertools
from functools import lru_cache

import numpy as np

import jax

jax.config.update("jax_compilation_cache_dir", "/tmp/jax_comp_cache")
jax.config.update("jax_persistent_cache_min_entry_size_bytes", -1)
jax.config.update("jax_persistent_cache_min_compile_time_secs", 0)

import ml_dtypes

import concourse.bass as bass
import concourse.mybir as mybir
import concourse.tile as tile
from concourse import bacc
from concourse.bass_utils import run_bass_kernel_spmd
from concourse.kernels.tile_matmul import matmul_tile_kernel

N, D, H, O, E = 8192, 1024, 2048, 1024, 8
NCORES = 8
TPC = N // NCORES  # tokens per core (gating / combine phases)
F32 = mybir.dt.float32
F32R = mybir.dt.float32r
BF = mybir.dt.bfloat16
NPBF = ml_dtypes.bfloat16
NSLOT = 3  # expert-MLP chains per core

_CACHE = {}


def _to_pmn(a):
    """[K, N] row-major -> [128, K/128, N] with row k = m*128 + p."""
    K, Nn = a.shape
    return np.ascontiguousarray(a.reshape(K // 128, 128, Nn).transpose(1, 0, 2))


def _nc():
    return bacc.Bacc(None, target_bir_lowering=False, debug=True)


def _build_gate_nc():
    # fp32r: full fp32 operands/accumulation, 1 PE cycle/row (vs 4 for
    # strict fp32) once the moving free dim is >=256. Keeps top-2
    # selection faithful to the fp32 reference (bf16 logits flip ~50
    # near-tie tokens, which blows the error budget).
    nc = _nc()
    xt = nc.dram_tensor("xt", [128, D // 128, TPC], F32R, kind="ExternalInput")
    wg1 = nc.dram_tensor("wg1", [128, D // 128, 128], F32R, kind="ExternalInput")
    wg2 = nc.dram_tensor("wg2", [128, 1, 128], F32R, kind="ExternalInput")
    g1t = nc.dram_tensor("g1t", [128, 1, TPC], F32R)
    logt = nc.dram_tensor("logt", [128, 1, TPC], F32, kind="ExternalOutput")
    with tile.TileContext(nc) as tc:
        matmul_tile_kernel(tc, wg1[:], xt[:], g1t[:], use_relu=True)
        matmul_tile_kernel(tc, wg2[:], g1t[:], logt[:])
    nc.compile()
    return nc


def _build_mlp_nc(sizes):
    # Custom fused 3-layer bf16 MLP, one chain per (core, slot). Each
    # chain carries its own expert's weights as per-core inputs, so chain
    # sizes can be packed to the actual routing counts (no 512-padding).
    # h1/h2 stay in SBUF (no HBM round trip). Weights arrive m-major
    # ([128, M/128, K/128, 128]) so each output tile's weights are one
    # contiguous DMA. Weights+x stream on the SP DMA queue (never blocked
    # behind compute-dependent writes); y writes go out on the Pool queue.
    nc = _nc()
    t = {}
    for j, s in enumerate(sizes):
        t[f"xt{j}"] = nc.dram_tensor(f"xt{j}", [128, D // 128, s], BF, kind="ExternalInput")
        t[f"w1{j}"] = nc.dram_tensor(f"w1{j}", [128, H // 128, D // 128, 128], BF, kind="ExternalInput")
        t[f"w2{j}"] = nc.dram_tensor(f"w2{j}", [128, H // 128, H // 128, 128], BF, kind="ExternalInput")
        t[f"w3{j}"] = nc.dram_tensor(f"w3{j}", [128, O // 128, H // 128, 128], BF, kind="ExternalInput")
        t[f"yt{j}"] = nc.dram_tensor(f"yt{j}", [128, O // 128, s], BF, kind="ExternalOutput")
    smax = max(sizes)
    Relu = mybir.ActivationFunctionType.Relu
    with tile.TileContext(nc) as tc:
        with (
            tc.tile_pool(name="wp", bufs=4) as wp,
            tc.tile_pool(name="xp", bufs=2) as xp,
            tc.tile_pool(name="hp", bufs=1) as hp,
            tc.tile_pool(name="yp", bufs=2) as yp,
            tc.tile_pool(name="pp", bufs=8, space="PSUM") as pp,
            tc.tile_pool(name="cst", bufs=1) as cst,
        ):
            bias = cst.tile([128, 1], F32, tag="bias")
            nc.any.memset(bias[:], 0.0)
            for j, s in enumerate(sizes):
                nch = -(-s // 512)
                chunk = -(-s // nch)
                fr = [(f0, min(chunk, s - f0)) for f0 in range(0, s, chunk)]
                xt_t = xp.tile([128, D // 128, smax], BF, tag="x", name="xt_t")[:, :, :s]
                for k in range(D // 128):
                    nc.sync.dma_start(xt_t[:, k, :], t[f"xt{j}"][:, k, :])
                h1_t = hp.tile([128, H // 128, smax], BF, tag="h1", name="h1_t")[:, :, :s]
                h2_t = hp.tile([128, H // 128, smax], BF, tag="h2", name="h2_t")[:, :, :s]
                y_t = yp.tile([128, O // 128, smax], BF, tag="y", name="y_t")[:, :, :s]

                def layer(wdram, in_t, out_t, K, M, relu, wtag):
                    for m in range(M):
                        wm = wp.tile([128, K, 128], BF, tag=wtag, name=f"wm_{wtag}")
                        nc.sync.dma_start(wm[:], wdram[:, m])
                        for f0, fsz in fr:
                            ps = pp.tile([128, 512], F32, tag="ps", name="ps")[:, :fsz]
                            for k in range(K):
                                nc.tensor.matmul(
                                    ps,
                                    wm[:, k, :],
                                    in_t[:, k, f0 : f0 + fsz],
                                    start=(k == 0),
                                    stop=(k == K - 1),
                                )
                            if relu:
                                nc.scalar.activation(
                                    out_t[:, m, f0 : f0 + fsz], ps, Relu, bias=bias[:]
                                )
                            else:
                                nc.any.tensor_copy(
                                    out=out_t[:, m, f0 : f0 + fsz], in_=ps
                                )

                layer(t[f"w1{j}"], xt_t, h1_t, D // 128, H // 128, True, "w1")
                layer(t[f"w2{j}"], h1_t, h2_t, H // 128, H // 128, True, "w2")
                layer(t[f"w3{j}"], h2_t, y_t, H // 128, O // 128, False, "w3")
                for m in range(O // 128):
                    nc.gpsimd.dma_start(t[f"yt{j}"][:, m, :], y_t[:, m, :])
    nc.compile()
    return nc


def _build_combine_nc(yall_rows):
    nc = _nc()
    ntiles = TPC // 128
    yall = nc.dram_tensor("yall", [yall_rows, O], BF, kind="ExternalInput")
    lg = nc.dram_tensor("lg", [128, ntiles, E], F32, kind="ExternalInput")
    m0 = nc.dram_tensor("m0", [128, ntiles, E], F32, kind="ExternalInput")
    m1 = nc.dram_tensor("m1", [128, ntiles, E], F32, kind="ExternalInput")
    i0 = nc.dram_tensor("i0", [128, ntiles], mybir.dt.int32, kind="ExternalInput")
    i1 = nc.dram_tensor("i1", [128, ntiles], mybir.dt.int32, kind="ExternalInput")
    out = nc.dram_tensor("out", [128, ntiles, O], F32, kind="ExternalOutput")
    X = mybir.AxisListType.X
    with tile.TileContext(nc) as tc:
        with (
            tc.tile_pool(name="big", bufs=4) as gp,
            tc.tile_pool(name="acc", bufs=4) as ap,
            tc.tile_pool(name="small", bufs=8) as sp,
            tc.tile_pool(name="idx", bufs=4) as ip,
        ):
            for i in range(ntiles):
                lg_t = sp.tile([128, E], F32, tag="lg")
                m0_t = sp.tile([128, E], F32, tag="m0")
                m1_t = sp.tile([128, E], F32, tag="m1")
                i0_t = ip.tile([128, 1], mybir.dt.int32, tag="i0")
                i1_t = ip.tile([128, 1], mybir.dt.int32, tag="i1")
                nc.sync.dma_start(lg_t[:], lg[:, i, :])
                nc.sync.dma_start(m0_t[:], m0[:, i, :])
                nc.sync.dma_start(m1_t[:], m1[:, i, :])
                nc.sync.dma_start(i0_t[:], i0[:, i : i + 1])
                nc.sync.dma_start(i1_t[:], i1[:, i : i + 1])

                g0 = gp.tile([128, O], BF, tag="g0")
                g1 = gp.tile([128, O], BF, tag="g1")
                nc.gpsimd.indirect_dma_start(
                    out=g0[:],
                    out_offset=None,
                    in_=yall[:],
                    in_offset=bass.IndirectOffsetOnAxis(ap=i0_t[:, :1], axis=0),
                )
                nc.gpsimd.indirect_dma_start(
                    out=g1[:],
                    out_offset=None,
                    in_=yall[:],
                    in_offset=bass.IndirectOffsetOnAxis(ap=i1_t[:, :1], axis=0),
                )

                rmax = sp.tile([128, 1], F32, tag="rmax")
                nc.vector.reduce_max(out=rmax[:], in_=lg_t[:], axis=X)
                ex = sp.tile([128, E], F32, tag="ex")
                nc.vector.tensor_scalar_sub(out=ex[:], in0=lg_t[:], scalar1=rmax[:])
                nc.scalar.activation(
                    out=ex[:], in_=ex[:], func=mybir.ActivationFunctionType.Exp
                )
                s = sp.tile([128, 1], F32, tag="s")
                nc.vector.reduce_sum(out=s[:], in_=ex[:], axis=X)
                inv = sp.tile([128, 1], F32, tag="inv")
                nc.vector.reciprocal(out=inv[:], in_=s[:])

                t0 = sp.tile([128, E], F32, tag="t0")
                nc.vector.tensor_mul(out=t0[:], in0=ex[:], in1=m0_t[:])
                w0 = sp.tile([128, 1], F32, tag="w0")
                nc.vector.reduce_sum(out=w0[:], in_=t0[:], axis=X)
                nc.vector.tensor_mul(out=w0[:], in0=w0[:], in1=inv[:])

                t1 = sp.tile([128, E], F32, tag="t1")
                nc.vector.tensor_mul(out=t1[:], in0=ex[:], in1=m1_t[:])
                w1v = sp.tile([128, 1], F32, tag="w1")
                nc.vector.reduce_sum(out=w1v[:], in_=t1[:], axis=X)
                nc.vector.tensor_mul(out=w1v[:], in0=w1v[:], in1=inv[:])

                a0 = ap.tile([128, O], F32, tag="a0")
                a1 = ap.tile([128, O], F32, tag="a1")
                nc.vector.tensor_scalar_mul(out=a0[:], in0=g0[:], scalar1=w0[:])
                nc.vector.tensor_scalar_mul(out=a1[:], in0=g1[:], scalar1=w1v[:])
                nc.vector.tensor_add(out=a0[:], in0=a0[:], in1=a1[:])
                nc.sync.dma_start(out[:, i, :], a0[:])
    nc.compile()
    return nc


def _pack_slots(counts, G=32, max_extra=4096):
    """Choose NSLOT chain sizes (8 bins of each across cores) and an
    expert->bin-count assignment covering `counts`, minimizing total
    capacity. Returns (sizes desc, {expert: (k_0..k_{NSLOT-1})})."""
    counts = [int(c) for c in counts]
    order = sorted(range(len(counts)), key=lambda e: -counts[e])
    total = sum(counts)
    lo = -(-total // (NCORES * G)) * G

    def try_pack(sizes):
        sizes_t = tuple(sizes)

        @lru_cache(maxsize=None)
        def rec(i, rem):
            if i == len(order):
                return ()
            c = counts[order[i]]
            if c == 0:
                sub = rec(i + 1, rem)
                return None if sub is None else ((0,) * NSLOT,) + sub
            for ks in itertools.product(*(range(r + 1) for r in rem)):
                cap = sum(k * s for k, s in zip(ks, sizes_t))
                if cap < c:
                    continue
                # any bin could be dropped if overshoot >= its size
                if cap - c >= max(sizes_t):
                    continue
                sub = rec(i + 1, tuple(r - k for r, k in zip(rem, ks)))
                if sub is not None:
                    return (ks,) + sub
            return None

        res = rec(0, (NCORES,) * NSLOT)
        if res is None:
            return None
        return {order[i]: res[i] for i in range(len(order))}

    for tot in range(lo, lo + max_extra + 1, G):
        for s1 in range(-(-tot // NSLOT // G) * G, tot - (NSLOT - 1) * G + 1, G):
            rem1 = tot - s1
            for s2 in range(-(-rem1 // (NSLOT - 1) // G) * G, min(s1, rem1 - G) + 1, G):
                s3 = rem1 - s2
                if s3 < G or s3 > s2:
                    continue
                asg = try_pack((s1, s2, s3))
                if asg is not None:
                    return (s1, s2, s3), asg
    # fallback: uniform bins
    S = G
    while sum(-(-c // S) for c in counts if c) > NCORES * NSLOT:
        S += G
    asg = try_pack((S, S, S))
    assert asg is not None
    return (S, S, S), asg


def kernel(x, W1, b1, W2, b2, W3, b3, Wg1, bg1, Wg2, bg2, top_k):
    x = np.asarray(x, np.float32)
    W1 = np.asarray(W1, np.float32)
    W2 = np.asarray(W2, np.float32)
    W3 = np.asarray(W3, np.float32)
    Wg1 = np.asarray(Wg1, np.float32)
    Wg2 = np.asarray(Wg2, np.float32)
    assert int(np.asarray(top_k)) == 2
    for b in (b1, b2, b3, bg1, bg2):
        assert not np.any(np.asarray(b)), "nonzero biases unsupported"

    core_ids = list(range(NCORES))

    # ---------------- L1: gating logits on device (fp32r) ----------------
    if "gate" not in _CACHE:
        _CACHE["gate"] = _build_gate_nc()
    nc1 = _CACHE["gate"]

    xT = np.ascontiguousarray(x.T)  # [D, N] fp32
    wg1p = np.zeros((D, 128), np.float32)
    wg1p[:, :64] = Wg1
    wg2p = np.zeros((128, 128), np.float32)
    wg2p[:64, :E] = Wg2
    wg1_pmn = _to_pmn(wg1p)
    wg2_pmn = _to_pmn(wg2p)
    in1 = [
        {
            "xt": _to_pmn(xT[:, c * TPC : (c + 1) * TPC]),
            "wg1": wg1_pmn,
            "wg2": wg2_pmn,
        }
        for c in core_ids
    ]
    res1 = run_bass_kernel_spmd(nc1, in1, core_ids).results
    logits = np.concatenate(
        [res1[c]["logt"][:E, 0, :].T for c in core_ids], axis=0
    )  # [N, E]

    # ---------------- host routing (comparisons/indexing only) ----------------
    top2 = np.argsort(-logits, axis=1, kind="stable")[:, :2]  # [N, 2]
    e0s, e1s = top2[:, 0], top2[:, 1]
    expert_lists = [np.nonzero((top2 == e).any(axis=1))[0] for e in range(E)]
    counts = [len(t) for t in expert_lists]

    sizes, asg = _pack_slots(counts)
    # ascending build order: smallest chain first minimizes the head
    # bubble (its x DMA gates the first matmul)
    perm = sorted(range(len(sizes)), key=lambda j: sizes[j])
    sizes = tuple(sizes[j] for j in perm)
    asg = {e: tuple(ks[j] for j in perm) for e, ks in asg.items()}
    percore = sum(sizes)

    # Materialize bins: bin slot j on core c covers yall rows
    # [c*percore + off_j, +sizes[j]). Assign each expert its bins
    # (largest-size first), filling each bin in token order.
    offs = np.concatenate([[0], np.cumsum(sizes)])[:NSLOT]
    bins_free = [list(range(NCORES)) for _ in range(NSLOT)]  # free cores per slot
    slot_expert = np.zeros((NCORES, NSLOT), np.int64)  # expert id per bin
    slot_tokens = [[np.zeros(0, np.int64)] * NSLOT for _ in range(NCORES)]
    glob = np.full((N, E), -1, np.int64)  # token,expert -> yall row
    for e in range(E):
        tl = expert_lists[e]
        p = 0
        for j in range(NSLOT):
            for _ in range(asg[e][j]):
                c = bins_free[j].pop(0)
                take = min(sizes[j], len(tl) - p)
                tok = tl[p : p + take]
                p += take
                slot_expert[c, j] = e
                slot_tokens[c][j] = tok
                glob[tok, e] = c * percore + offs[j] + np.arange(take)
        assert p == len(tl), f"packing failed for expert {e}"

    glob0 = glob[np.arange(N), e0s].astype(np.int32)
    glob1 = glob[np.arange(N), e1s].astype(np.int32)
    assert glob0.min() >= 0 and glob1.min() >= 0
    eye = np.eye(E, dtype=np.float32)

    # ---------------- L2: expert MLP on device (1 launch, bf16) ----------------
    key2 = ("mlp3", sizes)
    if key2 not in _CACHE:
        _CACHE[key2] = _build_mlp_nc(sizes)
    nc2 = _CACHE[key2]

    xT16 = xT.astype(NPBF)  # [D, N] bf16
    wcache = {}

    def _mmaj(a):
        # [128, K/128, M] pmn -> [128, M/128, K/128, 128] m-major
        P, K, M = a.shape
        return np.ascontiguousarray(
            a.reshape(P, K, M // 128, 128).transpose(0, 2, 1, 3)
        )

    def expert_w(e):
        if e not in wcache:
            wcache[e] = (
                _mmaj(_to_pmn(W1[e].astype(NPBF))),
                _mmaj(_to_pmn(W2[e].astype(NPBF))),
                _mmaj(_to_pmn(W3[e].astype(NPBF))),
            )
        return wcache[e]

    in2 = []
    for c in core_ids:
        m = {}
        for j, s in enumerate(sizes):
            tok = slot_tokens[c][j]
            xe = np.zeros((D, s), NPBF)
            xe[:, : len(tok)] = xT16[:, tok]
            w1p, w2p, w3p = expert_w(int(slot_expert[c, j]))
            m[f"xt{j}"] = _to_pmn(xe)
            m[f"w1{j}"] = w1p
            m[f"w2{j}"] = w2p
            m[f"w3{j}"] = w3p
        in2.append(m)
    res2 = run_bass_kernel_spmd(nc2, in2, core_ids).results

    yall = np.empty((NCORES * percore, O), NPBF)
    for c in core_ids:
        for j, s in enumerate(sizes):
            yT = res2[c][f"yt{j}"].transpose(1, 0, 2).reshape(O, s)
            r0 = c * percore + offs[j]
            yall[r0 : r0 + s] = yT.T

    # ---------------- L3: gather + softmax combine on device ----------------
    key3 = ("comb", NCORES * percore)
    if key3 not in _CACHE:
        _CACHE[key3] = _build_combine_nc(NCORES * percore)
    nc3 = _CACHE[key3]

    ntiles = TPC // 128

    def _pt(a):  # [TPC, ...] -> [128, ntiles, ...] with token = i*128 + p
        return np.ascontiguousarray(
            a.reshape(ntiles, 128, *a.shape[1:]).transpose(1, 0, *range(2, a.ndim + 1))
        )

    in3 = []
    for c in core_ids:
        sl = slice(c * TPC, (c + 1) * TPC)
        in3.append(
            {
                "yall": yall,
                "lg": _pt(logits[sl]),
                "m0": _pt(eye[e0s[sl]]),
                "m1": _pt(eye[e1s[sl]]),
                "i0": _pt(glob0[sl].reshape(TPC, 1))[:, :, 0],
                "i1": _pt(glob1[sl].reshape(TPC, 1))[:, :, 0],
            }
        )
    res3 = run_bass_kernel_spmd(nc3, in3, core_ids).results
    out = np.concatenate(
        [res3[c]["out"].transpose(1, 0, 2).reshape(TPC, O) for c in core_ids], axis=0
    )
    return out
